# revision 51
# baseline (speedup 1.0000x reference)
"""Trainium2 Bass kernel for the Mobius-addition broadcast problem.

out[m, n, :] = a[m,n] * B[n, :] + b[m,n] * x[m, :]
  with nB[n] = |B_n|^2, nx[m] = |x_m|^2, xy = x @ B^T,
       denom = 1 + 2*xy + nB*nx, a = (1 + 2*xy + nx)/denom,
       b = (1 - nB)/denom.

Sharding: data-parallel over M across 8 NeuronCores (M/8 = 256 rows each),
B replicated.  Per core the kernel is output-DMA-bound (128 MiB of fp32
writes at the 360 GB/s DMA roofline = 1456 ns per m-row), so every other
engine is kept under that budget:

  plane (prologue, ~12 us incl. PE-pstate warmup matmuls):
    natural b16[m,n] = (1-nB[n])/denom fp16 (xy matmul K=128 + K=2
    host-stacked-row accumulate [nx;1]x[nB;1], DVE recip + fp16-2x mul);
    transposed denomT -> rT = 1/denomT (DVE recip) and coefB_T -> cb
    (ACT PSUM->SBUF copy).  mb=1 of the b-plane is deferred into main-loop
    slack (first consumer is the chunk-16 staging DMA).
  main loop, per m (256 iters, steady-state 1456 ns = DMA-bound):
    - DVE: 8x fp16 tensor_scalar (4x perf mode, two chained f32
           per-partition scalars) ts = (B16 * cb_col) * rT_col
    - PE:  per 128-col region: K=1 fp16 matmul b16-row (x) x16-row into
           PSUM (start) + identity-matmul accumulate of ts (stop)
    - ACT: 2x [128,512] PSUM->SBUF fp32 copies
    - SP:  one 512 KiB slab DMA out; Pool: SWDGE staging of b16 rows
"""

import sys
from contextlib import ExitStack

import numpy as np

sys.path.insert(0, "/opt/trn_rl_repo")

import concourse.bacc as bacc  # noqa: E402
import concourse.bass as bass  # noqa: E402
import concourse.tile as tile  # noqa: E402
from concourse import mybir  # noqa: E402

N, M, D = 1024, 2048, 128
NCORES = 8
MC = M // NCORES  # 256 rows of x per core
F32 = mybir.dt.float32
F32R = mybir.dt.float32r
FP16 = mybir.dt.float16
ALU = mybir.AluOpType
CH = 8  # m rows per staged operand chunk


def _body(ctx, tc, out_d, bt16_d, x16_d, cxb16_d, snb_d, mc, n):
    nc = tc.nc
    nbs = n // 128       # n-blocks (8)
    mbs = mc // 128      # m-partition blocks (2)

    consts = ctx.enter_context(tc.tile_pool(name="consts", bufs=1))

    # ---- static inputs in SBUF (ordered by first use) ----
    # All loads on one queue, ordered by first use; the DMA-engine device is
    # exclusive, so issue order == landing order.
    # snall rows: [nB | nx | nx+1 ; ones ...]
    snall_sb = consts.tile([2, n + 2 * mc], FP16)
    nc.sync.dma_start(snall_sb[:], snb_d[:, :])
    snb_sb = snall_sb[:, 0:n]
    snx_sb = snall_sb[:, n:n + mc]
    nx1_sb = snall_sb[0:1, n + mc:n + 2 * mc]
    # bx16: [ 2*x.T | B.T ] fp16 -- split loads: the first covers what the
    # natural-plane h0 matmuls need, so they can start ~0.4us earlier.
    bx16_sb = consts.tile([128, n + mc], FP16)
    cut = mc + 512
    nc.sync.dma_start(bx16_sb[:, 0:cut], bt16_d[:, 0:cut])
    nc.sync.dma_start(bx16_sb[:, cut:], bt16_d[:, cut:])
    xT216_sb = bx16_sb[:, 0:mc]
    BT16_sb = bx16_sb[:, mc:mc + n]
    # pack16: [ cxb | B16-layout | eye ] fp16 (two DMAs: cxb lands first)
    pack16_sb = consts.tile([128, 2 * n + 128], FP16)
    nc.sync.dma_start(pack16_sb[:, 0:n], cxb16_d[:, 0:n])
    nc.sync.dma_start(pack16_sb[:, n:], cxb16_d[:, n:])
    cxb16_sb = pack16_sb[:, 0:n]
    B16_sb = pack16_sb[:, n:2 * n]
    eye16_sb = pack16_sb[:, 2 * n:2 * n + 128]
    x16_sb = consts.tile([1, mc * D], FP16)  # x rows flat on partition 0
    half = mc * D // 2  # two DMAs: a descriptor must be < 64 KiB
    nc.sync.dma_start(x16_sb[0:1, 0:half], x16_d[0:1, 0:half])
    nc.sync.dma_start(x16_sb[0:1, half:], x16_d[0:1, half:])
    ones1 = consts.tile([1, 128], FP16)
    nc.vector.memset(ones1[:], 1.0)
    warm16 = consts.tile([1, 128], FP16)
    nc.vector.memset(warm16[:], 0.0)

    # rT[p, g*512 + k*mc + m] = 1/denom[m, n], cb_sb same layout = coefB;
    # the main-loop ts op applies both as chained per-partition scalars.
    rT_sb = consts.tile([128, nbs * mc], F32)
    cb_sb = consts.tile([128, nbs * mc], F32)
    # b16 natural: [p, mb*n + j] = b[mb*128 + p, j]
    b16_sb = consts.tile([128, mbs * n], FP16)

    snbr = snb_sb
    snxr = snx_sb

    with ExitStack() as plane_ctx:
        ptmp = plane_ctx.enter_context(tc.tile_pool(name="ptmp", bufs=2))
        psum_pl = plane_ctx.enter_context(
            tc.tile_pool(name="psum_pl", bufs=2, space="PSUM"))
        psum_pa = plane_ctx.enter_context(
            tc.tile_pool(name="psum_pa", bufs=4, space="PSUM"))

        # PE warmup: keep the systolic array busy from t~1us so the plane
        # matmuls run at full pstate (2.4 GHz) instead of mid/low.  Rides a
        # psd-tagged slot (no consumers; freed before the b-plane needs it).
        psw = psum_pl.tile([128, 512], F32, tag="psd")
        for _ in range(26):
            nc.tensor.matmul(psw[:, 0:128], warm16[:], warm16[:],
                             start=True, stop=True)

        # ---- natural plane mb=0: b16[m-part, n-free] = (1-nB)/denom ----
        # (mb=1 is deferred into the main loop's slack; chunks 0-15 only
        # read mb=0 rows.)
        for h in range(2):
            hsl = slice(h * 512, (h + 1) * 512)
            psd = psum_pl.tile([128, 512], F32, tag="psd")
            nc.tensor.matmul(psd[:], xT216_sb[:, 0:128], BT16_sb[:, hsl],
                             start=True, stop=False)
            nc.tensor.matmul(psd[:], snxr[0:2, 0:128], snbr[0:2, hsl],
                             start=False, stop=True)
            r16 = ptmp.tile([128, 512], FP16, tag="r16")
            with nc.allow_low_precision(reason="fp16 ok: rel tol 2e-2"):
                nc.vector.reciprocal(r16[:], psd[:])
            nc.vector.tensor_mul(b16_sb[:, h * 512:(h + 1) * 512],
                                 r16[:], cxb16_sb[:, hsl])

        # ---- transposed plane: denom/coefB -> rT, cb_sb ----
        psa_l = []
        for g in range(nbs // 2):  # pairs of n-blocks share a PSUM tile
            psa = psum_pa.tile([128, 512], F32, tag="psa")
            psc = psum_pl.tile([128, 512], F32, tag="psc")
            psa_l.append(psa)
            for k in range(2):
                nb = g * 2 + k
                sl = slice(nb * 128, (nb + 1) * 128)
                ksl = slice(k * mc, (k + 1) * mc)
                nc.tensor.matmul(psa[:, ksl], BT16_sb[:, sl], xT216_sb[:],
                                 start=True, stop=False)
                nc.tensor.matmul(psa[:, ksl], snbr[0:2, sl], snxr[0:2, :],
                                 start=False, stop=True)
                nc.tensor.matmul(psc[:, ksl], BT16_sb[:, sl], xT216_sb[:],
                                 start=True, stop=False)
                nc.tensor.matmul(psc[:, ksl], ones1[:], nx1_sb[:],
                                 start=False, stop=True)
            nc.scalar.copy(cb_sb[:, g * 512:(g + 1) * 512], psc[:])
            nc.vector.reciprocal(rT_sb[:, g * 512:(g + 1) * 512], psa[:])

    # ---- main loop ----
    psum_main = ctx.enter_context(
        tc.tile_pool(name="psum_main", bufs=8, space="PSUM"))
    ts_pool = ctx.enter_context(tc.tile_pool(name="tsp", bufs=3))
    out_pool = ctx.enter_context(tc.tile_pool(name="outp", bufs=4))
    opch = ctx.enter_context(tc.tile_pool(name="opch", bufs=2))

    def stage(c):
        r0 = c * CH
        mb, prow = r0 // 128, r0 % 128
        bst = opch.tile([1, CH * n], FP16, tag="bst")
        nc.gpsimd.dma_start(bst[0:1, :],
                            b16_sb[prow:prow + CH, mb * n:(mb + 1) * n])
        return bst

    nxt = stage(0)
    for c in range(mc // CH):
        bst = nxt
        if c + 1 < mc // CH:
            nxt = stage(c + 1)
        for mloc in range(CH):
            m = c * CH + mloc
            if m in (40, 56):
                # deferred natural-plane mb=1 in the main loop's slack:
                # first consumer is the chunk-16 staging DMA, ~200 us away.
                # (GPSIMD cannot read PSUM, so the divides ride DVE slack.)
                for h in ((0,) if m == 40 else (1,)):
                    hsl = slice(h * 512, (h + 1) * 512)
                    psd1 = psum_main.tile([128, 512], F32, tag="pom")
                    nc.tensor.matmul(psd1[:], xT216_sb[:, 128:256],
                                     BT16_sb[:, hsl], start=True, stop=False)
                    nc.tensor.matmul(psd1[:], snxr[0:2, 128:256],
                                     snbr[0:2, hsl], start=False, stop=True)
                    r16b = ts_pool.tile([128, 512], FP16, tag="r16b")
                    with nc.allow_low_precision(reason="fp16 tol 2e-2"):
                        nc.vector.reciprocal(r16b[:], psd1[:])
                    nc.vector.tensor_mul(
                        b16_sb[:, n + h * 512: n + (h + 1) * 512],
                        r16b[:], cxb16_sb[:, hsl])
            ts16 = ts_pool.tile([128, n], FP16, tag="ts")
            for nb in range(nbs):
                sl = slice(nb * 128, (nb + 1) * 128)
                col = slice(nb * mc + m, nb * mc + m + 1)
                nc.vector.tensor_scalar(
                    ts16[:, sl], B16_sb[:, sl],
                    cb_sb[:, col], rT_sb[:, col],
                    op0=ALU.mult, op1=ALU.mult)
            pos = [psum_main.tile([128, 512], F32, tag="pom", name=f"pom{g}")
                   for g in range(2)]
            for nb in range(nbs):
                g, go = nb // 4, (nb % 4) * 128
                nc.tensor.matmul(
                    pos[g][:, go:go + 128],
                    bst[0:1, mloc * n + nb * 128: mloc * n + (nb + 1) * 128],
                    x16_sb[0:1, m * D:(m + 1) * D],
                    start=True, stop=False)
                nc.tensor.matmul(
                    pos[g][:, go:go + 128], eye16_sb[:],
                    ts16[:, nb * 128:(nb + 1) * 128],
                    start=False, stop=True)
            ot = out_pool.tile([128, n], F32, tag="ot")
            for g in range(2):
                nc.scalar.copy(ot[:, g * 512:(g + 1) * 512], pos[g][:])
            slab = out_d[m * n:(m + 1) * n, :].rearrange(
                "(nb p) d -> p nb d", p=128)
            nc.sync.dma_start(slab, ot[:])


def build_program(mc=MC, n=N):
    nc = bacc.Bacc("TRN2", target_bir_lowering=False, debug=False,
                   num_devices=NCORES)
    bt16_d = nc.dram_tensor("bt16_in", [D, n + mc], FP16,
                            kind="ExternalInput").ap()
    x16_d = nc.dram_tensor("x16_in", [1, mc * D], FP16,
                           kind="ExternalInput").ap()
    cxb16_d = nc.dram_tensor("cxb16_in", [128, 2 * n + 128], FP16,
                             kind="ExternalInput").ap()
    snb_d = nc.dram_tensor("snb_in", [2, n + 2 * mc], FP16,
                           kind="ExternalInput").ap()
    out_d = nc.dram_tensor("out", [mc * n, D], F32, kind="ExternalOutput").ap()
    with tile.TileContext(nc) as tc:
        with ExitStack() as ctx:
            _body(ctx, tc, out_d, bt16_d, x16_d, cxb16_d, snb_d, mc, n)
    nc.compile()
    return nc


_NC_CACHE = None


def _get_nc():
    global _NC_CACHE
    if _NC_CACHE is None:
        _NC_CACHE = build_program()
    return _NC_CACHE


def make_in_maps(B, x):
    B = np.ascontiguousarray(np.asarray(B, dtype=np.float32))
    x = np.ascontiguousarray(np.asarray(x, dtype=np.float32))
    f16 = np.float16
    nB = np.sum(B * B, axis=1)                      # [N]
    b16l = (B.astype(f16).reshape(N // 128, 128, D).transpose(1, 0, 2)
            .reshape(128, N))
    cxb16 = np.broadcast_to((1.0 - nB).astype(f16)[None, :], (128, N))
    eye16 = np.eye(128, dtype=f16)
    # pack16 = [ cxb | B16-layout | eye ]
    pack16 = np.ascontiguousarray(np.concatenate([cxb16, b16l, eye16], axis=1))
    in_maps = []
    for c in range(NCORES):
        xs = np.ascontiguousarray(x[c * MC:(c + 1) * MC])
        nx = np.sum(xs * xs, axis=1)                # [MC]
        # snall rows: [nB | nx | nx+1 ; ones]
        snall = np.ones((2, N + 2 * MC), np.float16)
        snall[0, :N] = nB
        snall[0, N:N + MC] = nx
        snall[0, N + MC:] = nx + 1.0
        snall[1, :N] = 1.0
        # bx16 = [ B.T | 2*x.T ]
        bx16 = np.ascontiguousarray(np.concatenate(
            [(2.0 * xs.T).astype(f16), B.T.astype(f16)], axis=1))
        in_maps.append({
            "bt16_in": bx16,
            "x16_in": np.ascontiguousarray(xs.astype(f16).reshape(1, -1)),
            "cxb16_in": pack16,
            "snb_in": snall,
        })
    return in_maps


def kernel(B, x):
    from concourse.bass_utils import run_bass_kernel_spmd
    nc = _get_nc()
    in_maps = make_in_maps(B, x)
    res = run_bass_kernel_spmd(nc, in_maps, list(range(NCORES)))
    outs = [np.asarray(res.results[c]["out"]).reshape(MC, N, D)
            for c in range(NCORES)]
    return np.concatenate(outs, axis=0)


# revision 56
# speedup vs baseline: 1.0003x; 1.0003x over previous
"""Trainium2 Bass kernel for the Mobius-addition broadcast problem.

out[m, n, :] = a[m,n] * B[n, :] + b[m,n] * x[m, :]
  with nB[n] = |B_n|^2, nx[m] = |x_m|^2, xy = x @ B^T,
       denom = 1 + 2*xy + nB*nx, a = (1 + 2*xy + nx)/denom,
       b = (1 - nB)/denom.

Sharding: data-parallel over M across 8 NeuronCores (M/8 = 256 rows each),
B replicated.  Per core the kernel is output-DMA-bound (128 MiB of fp32
writes at the 360 GB/s DMA roofline = 1456 ns per m-row), so every other
engine is kept under that budget:

  plane (prologue, ~12 us incl. PE-pstate warmup matmuls):
    natural b16[m,n] = (1-nB[n])/denom fp16 (xy matmul K=128 + K=2
    host-stacked-row accumulate [nx;1]x[nB;1], DVE recip + fp16-2x mul);
    transposed denomT -> rT = 1/denomT (DVE recip) and coefB_T -> cb
    (ACT PSUM->SBUF copy).  mb=1 of the b-plane is deferred into main-loop
    slack (first consumer is the chunk-16 staging DMA).
  main loop, per m (256 iters, steady-state 1456 ns = DMA-bound):
    - DVE: 8x fp16 tensor_scalar (4x perf mode, two chained f32
           per-partition scalars) ts = (B16 * cb_col) * rT_col
    - PE:  per 128-col region: K=1 fp16 matmul b16-row (x) x16-row into
           PSUM (start) + identity-matmul accumulate of ts (stop)
    - ACT: 2x [128,512] PSUM->SBUF fp32 copies
    - SP:  one 512 KiB slab DMA out; Pool: SWDGE staging of b16 rows
"""

import sys
from contextlib import ExitStack

import numpy as np

sys.path.insert(0, "/opt/trn_rl_repo")

import concourse.bacc as bacc  # noqa: E402
import concourse.bass as bass  # noqa: E402
import concourse.tile as tile  # noqa: E402
from concourse import mybir  # noqa: E402

N, M, D = 1024, 2048, 128
NCORES = 8
MC = M // NCORES  # 256 rows of x per core
F32 = mybir.dt.float32
F32R = mybir.dt.float32r
FP16 = mybir.dt.float16
ALU = mybir.AluOpType
CH = 16  # m rows per staged operand chunk


def _body(ctx, tc, out_d, bt16_d, x16_d, cxb16_d, snb_d, mc, n):
    nc = tc.nc
    nbs = n // 128       # n-blocks (8)
    mbs = mc // 128      # m-partition blocks (2)

    consts = ctx.enter_context(tc.tile_pool(name="consts", bufs=1))

    # ---- static inputs in SBUF (ordered by first use) ----
    # All loads on one queue, ordered by first use; the DMA-engine device is
    # exclusive, so issue order == landing order.
    # snall rows: [nB | nx | nx+1 ; ones ...]
    snall_sb = consts.tile([2, n + 2 * mc], FP16)
    nc.sync.dma_start(snall_sb[:], snb_d[:, :])
    snb_sb = snall_sb[:, 0:n]
    snx_sb = snall_sb[:, n:n + mc]
    nx1_sb = snall_sb[0:1, n + mc:n + 2 * mc]
    # bx16: [ 2*x.T | B.T ] fp16 -- split loads: the first covers what the
    # natural-plane h0 matmuls need, so they can start ~0.4us earlier.
    bx16_sb = consts.tile([128, n + mc], FP16)
    cut = mc + 512
    nc.sync.dma_start(bx16_sb[:, 0:cut], bt16_d[:, 0:cut])
    nc.sync.dma_start(bx16_sb[:, cut:], bt16_d[:, cut:])
    xT216_sb = bx16_sb[:, 0:mc]
    BT16_sb = bx16_sb[:, mc:mc + n]
    # pack16: [ cxb | B16-layout | eye ] fp16 (two DMAs: cxb lands first)
    pack16_sb = consts.tile([128, 2 * n + 128], FP16)
    nc.sync.dma_start(pack16_sb[:, 0:n], cxb16_d[:, 0:n])
    nc.sync.dma_start(pack16_sb[:, n:], cxb16_d[:, n:])
    cxb16_sb = pack16_sb[:, 0:n]
    B16_sb = pack16_sb[:, n:2 * n]
    eye16_sb = pack16_sb[:, 2 * n:2 * n + 128]
    x16_sb = consts.tile([1, mc * D], FP16)  # x rows flat on partition 0
    half = mc * D // 2  # two DMAs: a descriptor must be < 64 KiB
    nc.sync.dma_start(x16_sb[0:1, 0:half], x16_d[0:1, 0:half])
    nc.sync.dma_start(x16_sb[0:1, half:], x16_d[0:1, half:])
    ones1 = consts.tile([1, 128], FP16)
    nc.vector.memset(ones1[:], 1.0)
    warm16 = consts.tile([1, 128], FP16)
    nc.vector.memset(warm16[:], 0.0)

    # rT[p, g*512 + k*mc + m] = 1/denom[m, n], cb_sb same layout = coefB;
    # the main-loop ts op applies both as chained per-partition scalars.
    rT_sb = consts.tile([128, nbs * mc], F32)
    cb_sb = consts.tile([128, nbs * mc], F32)
    # b16 natural: [p, mb*n + j] = b[mb*128 + p, j]
    b16_sb = consts.tile([128, mbs * n], FP16)

    snbr = snb_sb
    snxr = snx_sb

    with ExitStack() as plane_ctx:
        ptmp = plane_ctx.enter_context(tc.tile_pool(name="ptmp", bufs=2))
        psum_pl = plane_ctx.enter_context(
            tc.tile_pool(name="psum_pl", bufs=2, space="PSUM"))
        psum_pa = plane_ctx.enter_context(
            tc.tile_pool(name="psum_pa", bufs=4, space="PSUM"))

        # PE warmup: keep the systolic array busy from t~1us so the plane
        # matmuls run at full pstate (2.4 GHz) instead of mid/low.  Rides a
        # psd-tagged slot (no consumers; freed before the b-plane needs it).
        psw = psum_pl.tile([128, 512], F32, tag="psd")
        for _ in range(26):
            nc.tensor.matmul(psw[:, 0:128], warm16[:], warm16[:],
                             start=True, stop=True)

        # ---- natural plane mb=0: b16[m-part, n-free] = (1-nB)/denom ----
        # (mb=1 is deferred into the main loop's slack; chunks 0-15 only
        # read mb=0 rows.)
        for h in range(2):
            hsl = slice(h * 512, (h + 1) * 512)
            psd = psum_pl.tile([128, 512], F32, tag="psd")
            nc.tensor.matmul(psd[:], xT216_sb[:, 0:128], BT16_sb[:, hsl],
                             start=True, stop=False)
            nc.tensor.matmul(psd[:], snxr[0:2, 0:128], snbr[0:2, hsl],
                             start=False, stop=True)
            r16 = ptmp.tile([128, 512], FP16, tag="r16")
            with nc.allow_low_precision(reason="fp16 ok: rel tol 2e-2"):
                nc.vector.reciprocal(r16[:], psd[:])
            nc.vector.tensor_mul(b16_sb[:, h * 512:(h + 1) * 512],
                                 r16[:], cxb16_sb[:, hsl])

        # ---- transposed plane: denom/coefB -> rT, cb_sb ----
        psa_l = []
        for g in range(nbs // 2):  # pairs of n-blocks share a PSUM tile
            psa = psum_pa.tile([128, 512], F32, tag="psa")
            psc = psum_pl.tile([128, 512], F32, tag="psc")
            psa_l.append(psa)
            for k in range(2):
                nb = g * 2 + k
                sl = slice(nb * 128, (nb + 1) * 128)
                ksl = slice(k * mc, (k + 1) * mc)
                nc.tensor.matmul(psa[:, ksl], BT16_sb[:, sl], xT216_sb[:],
                                 start=True, stop=False)
                nc.tensor.matmul(psa[:, ksl], snbr[0:2, sl], snxr[0:2, :],
                                 start=False, stop=True)
                nc.tensor.matmul(psc[:, ksl], BT16_sb[:, sl], xT216_sb[:],
                                 start=True, stop=False)
                nc.tensor.matmul(psc[:, ksl], ones1[:], nx1_sb[:],
                                 start=False, stop=True)
            nc.scalar.copy(cb_sb[:, g * 512:(g + 1) * 512], psc[:])
            nc.vector.reciprocal(rT_sb[:, g * 512:(g + 1) * 512], psa[:])

    # ---- main loop ----
    psum_main = ctx.enter_context(
        tc.tile_pool(name="psum_main", bufs=8, space="PSUM"))
    ts_pool = ctx.enter_context(tc.tile_pool(name="tsp", bufs=4))
    out_pool = ctx.enter_context(tc.tile_pool(name="outp", bufs=6))
    opch = ctx.enter_context(tc.tile_pool(name="opch", bufs=2))

    def stage(c):
        r0 = c * CH
        mb, prow = r0 // 128, r0 % 128
        bst = opch.tile([1, CH * n], FP16, tag="bst")
        nc.gpsimd.dma_start(bst[0:1, :],
                            b16_sb[prow:prow + CH, mb * n:(mb + 1) * n])
        return bst

    nxt = stage(0)
    for c in range(mc // CH):
        bst = nxt
        if c + 1 < mc // CH:
            nxt = stage(c + 1)
        for mloc in range(CH):
            m = c * CH + mloc
            if m in (24, 48):
                # deferred natural-plane mb=1 in the main loop's slack:
                # first consumer is the chunk-16 staging DMA, ~200 us away.
                # (GPSIMD cannot read PSUM, so the divides ride DVE slack.)
                for h in ((0,) if m == 24 else (1,)):
                    hsl = slice(h * 512, (h + 1) * 512)
                    psd1 = psum_main.tile([128, 512], F32, tag="pom")
                    nc.tensor.matmul(psd1[:], xT216_sb[:, 128:256],
                                     BT16_sb[:, hsl], start=True, stop=False)
                    nc.tensor.matmul(psd1[:], snxr[0:2, 128:256],
                                     snbr[0:2, hsl], start=False, stop=True)
                    r16b = ts_pool.tile([128, 512], FP16, tag="r16b")
                    with nc.allow_low_precision(reason="fp16 tol 2e-2"):
                        nc.vector.reciprocal(r16b[:], psd1[:])
                    nc.vector.tensor_mul(
                        b16_sb[:, n + h * 512: n + (h + 1) * 512],
                        r16b[:], cxb16_sb[:, hsl])
            ts16 = ts_pool.tile([128, n], FP16, tag="ts")
            for nb in range(nbs):
                sl = slice(nb * 128, (nb + 1) * 128)
                col = slice(nb * mc + m, nb * mc + m + 1)
                nc.vector.tensor_scalar(
                    ts16[:, sl], B16_sb[:, sl],
                    cb_sb[:, col], rT_sb[:, col],
                    op0=ALU.mult, op1=ALU.mult)
            pos = [psum_main.tile([128, 512], F32, tag="pom", name=f"pom{g}")
                   for g in range(2)]
            for nb in range(nbs):
                g, go = nb // 4, (nb % 4) * 128
                nc.tensor.matmul(
                    pos[g][:, go:go + 128],
                    bst[0:1, mloc * n + nb * 128: mloc * n + (nb + 1) * 128],
                    x16_sb[0:1, m * D:(m + 1) * D],
                    start=True, stop=False)
                nc.tensor.matmul(
                    pos[g][:, go:go + 128], eye16_sb[:],
                    ts16[:, nb * 128:(nb + 1) * 128],
                    start=False, stop=True)
            ot = out_pool.tile([128, n], F32, tag="ot")
            for g in range(2):
                nc.scalar.copy(ot[:, g * 512:(g + 1) * 512], pos[g][:])
            slab = out_d[m * n:(m + 1) * n, :].rearrange(
                "(nb p) d -> p nb d", p=128)
            nc.sync.dma_start(slab, ot[:])


def build_program(mc=MC, n=N):
    nc = bacc.Bacc("TRN2", target_bir_lowering=False, debug=False,
                   num_devices=NCORES)
    bt16_d = nc.dram_tensor("bt16_in", [D, n + mc], FP16,
                            kind="ExternalInput").ap()
    x16_d = nc.dram_tensor("x16_in", [1, mc * D], FP16,
                           kind="ExternalInput").ap()
    cxb16_d = nc.dram_tensor("cxb16_in", [128, 2 * n + 128], FP16,
                             kind="ExternalInput").ap()
    snb_d = nc.dram_tensor("snb_in", [2, n + 2 * mc], FP16,
                           kind="ExternalInput").ap()
    out_d = nc.dram_tensor("out", [mc * n, D], F32, kind="ExternalOutput").ap()
    with tile.TileContext(nc) as tc:
        with ExitStack() as ctx:
            _body(ctx, tc, out_d, bt16_d, x16_d, cxb16_d, snb_d, mc, n)
    nc.compile()
    return nc


_NC_CACHE = None


def _get_nc():
    global _NC_CACHE
    if _NC_CACHE is None:
        _NC_CACHE = build_program()
    return _NC_CACHE


def make_in_maps(B, x):
    B = np.ascontiguousarray(np.asarray(B, dtype=np.float32))
    x = np.ascontiguousarray(np.asarray(x, dtype=np.float32))
    f16 = np.float16
    nB = np.sum(B * B, axis=1)                      # [N]
    b16l = (B.astype(f16).reshape(N // 128, 128, D).transpose(1, 0, 2)
            .reshape(128, N))
    cxb16 = np.broadcast_to((1.0 - nB).astype(f16)[None, :], (128, N))
    eye16 = np.eye(128, dtype=f16)
    # pack16 = [ cxb | B16-layout | eye ]
    pack16 = np.ascontiguousarray(np.concatenate([cxb16, b16l, eye16], axis=1))
    in_maps = []
    for c in range(NCORES):
        xs = np.ascontiguousarray(x[c * MC:(c + 1) * MC])
        nx = np.sum(xs * xs, axis=1)                # [MC]
        # snall rows: [nB | nx | nx+1 ; ones]
        snall = np.ones((2, N + 2 * MC), np.float16)
        snall[0, :N] = nB
        snall[0, N:N + MC] = nx
        snall[0, N + MC:] = nx + 1.0
        snall[1, :N] = 1.0
        # bx16 = [ B.T | 2*x.T ]
        bx16 = np.ascontiguousarray(np.concatenate(
            [(2.0 * xs.T).astype(f16), B.T.astype(f16)], axis=1))
        in_maps.append({
            "bt16_in": bx16,
            "x16_in": np.ascontiguousarray(xs.astype(f16).reshape(1, -1)),
            "cxb16_in": pack16,
            "snb_in": snall,
        })
    return in_maps


def kernel(B, x):
    from concourse.bass_utils import run_bass_kernel_spmd
    nc = _get_nc()
    in_maps = make_in_maps(B, x)
    res = run_bass_kernel_spmd(nc, in_maps, list(range(NCORES)))
    outs = [np.asarray(res.results[c]["out"]).reshape(MC, N, D)
            for c in range(NCORES)]
    return np.concatenate(outs, axis=0)
